# revision 12
# baseline (speedup 1.0000x reference)
"""Trainium2 Bass kernel for blockwise soft-quantization dequant + codebook entropy.

Problem (nn_BlockwiseQuantizationOptim): weight [1024,1024] f32 split into 64
blocks of 128x128; per-block soft quantization onto 256 uniform levels with
softmax temperature T=100, returning (dequantized weight [1024,1024], total
soft-assignment entropy scalar).

Sharding: block axis across 8 cores (8 blocks/core). No collectives needed —
entropy is a per-block sum; host adds the 8 partial scalars and reassembles
the dequant blocks.

Math (closed forms instead of the naive [*, 256] softmax):
  With q_l = l/255, d = T/255, r = e^-d, the softmax over levels of
  -T|x - q_l| has geometric structure. For y = 255*clip(x_norm,0,1),
  t = y mod 1, k = y - t:
    ea = r^t, eb = r^(1-t), E1 = r^(k+1), E2 = r^(255-k)
    den = ea+eb - (ea*E1 + eb*E2)                    (= Z*(1-r))
    num = c1*(k*(ea+eb) + eb) + c3*(ea*E1/r - ea + eb) - c4*eb*E2
    w_q = (1/255) * num/den, with c1=1/(1-r), c3=r/(1-r)^2,
    c4=(256-255r)/(1-r)^2   (num here folded by 1/c1; see code)
  Entropy bin masses bin[j,l] = sum_i p_l(x[i,j]) reduce to per-column
  weighted histograms over the integer level k:
    HL[j,m] = sum_{i: k=m} ea/Z,  HR[j,m] = sum_{i: k=m} eb/Z
    bin = HL @ KL + HR @ KR,  KL[m,l] = r^(m-l)[m>=l], KR[m,l]=r^(l-1-m)[m<l]
  The histograms are built exactly on the PE: per column j a one-hot
  matrix O[i,m] = [k[i,j]==m] (bf16) is multiplied against (gl,gr).
"""
import sys

sys.path.insert(0, "/opt/trn_rl_repo")

import numpy as np

N_CORES = 8
BS = 128
NB_R = NB_C = 8
NB = NB_R * NB_C            # 64 blocks
BPC = NB // N_CORES         # 8 blocks per core
L = 256
T = 100.0
EPS = 1e-6
DELTA = 1.0 / (L - 1)
TD = T * DELTA              # 100/255

_cache = {}


def _constants():
    r = float(np.exp(np.float64(-TD)))
    c1 = 1.0 / (1.0 - r)
    c3 = r / (1.0 - r) ** 2
    c4 = (256.0 - 255.0 * r) / (1.0 - r) ** 2
    return r, c1, c3, c4


def _host_tables():
    """Digit-scheme tables:
    iotafb [128, 16, 128] bf16: value = b at [:, b, :]
    kdig   [16, 64, 128] bf16: kdig[b, (s*2+h)*16+a, ll] = K_s[16a+b, 128h+ll]
    """
    import ml_dtypes

    r, c1, _, _ = _constants()
    bf = ml_dtypes.bfloat16
    iotafb = np.broadcast_to(
        np.arange(16, dtype=np.float32)[None, :, None], (BS, 16, BS)
    ).astype(bf)
    m = np.arange(L, dtype=np.float64)[:, None]
    l = np.arange(L, dtype=np.float64)[None, :]
    KL = np.where(m >= l, np.exp(-TD * (m - l)), 0.0) * (1.0 - r)
    KR = np.where(m <= l - 1, np.exp(-TD * (l - 1 - m)), 0.0) * (1.0 - r)
    Ks = [KL.astype(np.float32), KR.astype(np.float32)]
    kdig = np.empty((16, 64, BS), dtype=np.float32)
    for s in range(2):
        for h in range(2):
            for a in range(16):
                for b_ in range(16):
                    kdig[b_, (s * 2 + h) * 16 + a, :] = Ks[s][
                        16 * a + b_, 128 * h : 128 * (h + 1)
                    ]
    return iotafb, kdig.astype(bf)


def _build_bass():
    import bass_rust
    import concourse.bass as bass
    import concourse.tile as tile
    import concourse.mybir as mybir
    from concourse.vector_clock import ScopedClock

    f32 = mybir.dt.float32
    bf16 = mybir.dt.bfloat16
    Alu = mybir.AluOpType
    Act = mybir.ActivationFunctionType
    r, c1, c3, c4 = _constants()

    # walrus's codegen accepts at most ONE sync wait per instruction.  Tile
    # emits several.  Fix up after scheduling: (a) drop waits on the
    # instruction's own engine semaphore — those engines complete in order,
    # so program order implies them; (b) hoist remaining extra waits onto
    # single-wait NoOps inserted just before the instruction on the same
    # engine stream.
    _self_pref = {
        "EngineType.Activation": "Activation_",
        "EngineType.DVE": "DVE_",
        "EngineType.PE": "PE_",
        "EngineType.SP": "SP_",
    }

    def _fix_sync_waits(nc):
        ctr = [0]
        for bb in nc.m.functions[0].blocks:
            out = []
            changed = False
            for inst in bb.instructions:
                si = inst.sync_info
                waits = list(si.on_wait) if si is not None and si.on_wait else []
                orig_n = len(waits)
                if len(waits) > 1:
                    pref = _self_pref.get(str(inst.engine))
                    if pref is not None:
                        kept = [
                            w
                            for w in waits
                            if not (
                                w.sync_type == "semaphore"
                                and w.ant_name
                                and w.ant_name.startswith(pref)
                            )
                        ]
                        waits = kept if kept else waits[-1:]
                if len(waits) > 1:
                    for w in waits[:-1]:
                        nop = bass_rust.InstNoOp(
                            name=f"I-wsplit-{ctr[0]}", ins=[], outs=[]
                        )
                        ctr[0] += 1
                        nop.engine = inst.engine
                        nop.sync_info = bass_rust.SyncInfo(
                            on_wait=[w], on_update=[]
                        )
                        out.append(nop)
                    waits = waits[-1:]
                if si is not None and len(waits) != orig_n:
                    inst.sync_info = bass_rust.SyncInfo(
                        on_wait=waits,
                        on_update=list(si.on_update) if si.on_update else [],
                    )
                    changed = True
                out.append(inst)
            if changed or len(out) != len(bb.instructions):
                bb.instructions = out

    nc = bass.Bass("TRN2", target_bir_lowering=False, debug=False)

    # register const APs for activation biases (only 0.0/1.0 pre-registered)
    for _cv in (-TD, -T, EPS):
        _ct = nc.alloc_sbuf_tensor(f"constx-f32-{_cv}", [128, 1], f32)
        nc.gpsimd.memset(_ct.ap(), _cv)
        nc.const_aps.aps[(f32, _cv)] = _ct.ap()
    nc.all_engine_barrier()

    wb = nc.dram_tensor("wb", [BPC, BS, BS], f32, kind="ExternalInput")
    wmn = nc.dram_tensor("wmn", [1, BPC], f32, kind="ExternalInput")
    wmx = nc.dram_tensor("wmx", [1, BPC], f32, kind="ExternalInput")
    iota_d = nc.dram_tensor("iotafb", [BS, 16, BS], bf16, kind="ExternalInput")
    kmat_d = nc.dram_tensor("kdig", [16, 64, BS], bf16, kind="ExternalInput")
    deq_d = nc.dram_tensor("deq", [BPC, BS, BS], f32, kind="ExternalOutput")
    ent_d = nc.dram_tensor("ent", [1, 1], f32, kind="ExternalOutput")

    with tile.TileContext(nc) as tc:
        with (
            tc.tile_pool(name="const", bufs=1) as cpool,
            tc.tile_pool(name="pre", bufs=1) as prepool,
            tc.tile_pool(name="work", bufs=3) as work,
            tc.tile_pool(name="onehot", bufs=3) as ohpool,
            tc.tile_pool(name="hbufs", bufs=1) as hbpool,
            tc.tile_pool(name="psum_hist", bufs=3, space="PSUM") as psum_hist,
            tc.tile_pool(name="psum_bin", bufs=1, space="PSUM") as psum_bin,
            tc.tile_pool(name="psum_small", bufs=1, space="PSUM") as psum_small,
        ):
            iota_t = cpool.tile([BS, 16, BS], bf16, tag="iota")
            nc.sync.dma_start(iota_t[:], iota_d.ap())
            kmat_t = cpool.tile([16, 64, BS], bf16, tag="kmat")
            nc.sync.dma_start(kmat_t[:], kmat_d.ap())
            ones_t = cpool.tile([BS, BS], f32, tag="ones")
            nc.vector.memset(ones_t[:], 1.0)

            # ---- per-block scalar prep, broadcast to all 128 partitions ----
            # (partition broadcast = ones-matmul on the PE)
            mnb = prepool.tile([1, BPC], f32, tag="mnb")
            mxb = prepool.tile([1, BPC], f32, tag="mxb")
            nc.sync.dma_start(mnb[:], wmn.ap())
            nc.sync.dma_start(mxb[:], wmx.ap())
            bc_mn = prepool.tile([BS, BPC], f32, tag="bcmn")
            bc_mx = prepool.tile([BS, BPC], f32, tag="bcmx")
            bc_ps = psum_small.tile([BS, BPC], f32, tag="small")
            nc.tensor.matmul(bc_ps[:], ones_t[0:1, :], mnb[:], start=True, stop=True)
            nc.vector.tensor_copy(bc_mn[:], bc_ps[:])
            bc_ps2 = psum_small.tile([BS, BPC], f32, tag="small")
            nc.tensor.matmul(bc_ps2[:], ones_t[0:1, :], mxb[:], start=True, stop=True)
            nc.vector.tensor_copy(bc_mx[:], bc_ps2[:])

            tmp_a = prepool.tile([BS, BPC], f32, tag="tmpa")
            wminv = prepool.tile([BS, BPC], f32, tag="wminv")
            wmaxv = prepool.tile([BS, BPC], f32, tag="wmaxv")
            scalev = prepool.tile([BS, BPC], f32, tag="scalev")
            invv = prepool.tile([BS, BPC], f32, tag="invv")
            sc255 = prepool.tile([BS, BPC], f32, tag="sc255")
            b255 = prepool.tile([BS, BPC], f32, tag="b255")
            dscalev = prepool.tile([BS, BPC], f32, tag="dscalev")

            nc.vector.tensor_scalar(tmp_a[:], bc_mx[:], EPS, None, Alu.subtract)
            nc.vector.tensor_tensor(wminv[:], bc_mn[:], tmp_a[:], Alu.min)
            nc.vector.tensor_scalar(tmp_a[:], wminv[:], EPS, None, Alu.add)
            nc.vector.tensor_tensor(wmaxv[:], bc_mx[:], tmp_a[:], Alu.max)
            nc.vector.tensor_sub(scalev[:], wmaxv[:], wminv[:])
            nc.vector.tensor_scalar(tmp_a[:], scalev[:], EPS, None, Alu.add)
            nc.vector.reciprocal(invv[:], tmp_a[:])
            nc.vector.tensor_scalar(sc255[:], invv[:], 255.0, None, Alu.mult)
            nc.vector.tensor_scalar(tmp_a[:], wminv[:], -255.0, None, Alu.mult)
            nc.vector.tensor_mul(b255[:], tmp_a[:], invv[:])
            nc.vector.tensor_scalar(dscalev[:], scalev[:], float(DELTA), None, Alu.mult)

            # entropy accumulator [128, 2*BPC], one column per (block, half)
            ent_acc = prepool.tile([BS, 2 * BPC], f32, tag="entacc")
            nc.vector.memset(ent_acc[:], 0.0)

            hb_all = hbpool.tile(
                [16, BPC, BS, 2, 16], bf16, tag="hball", name="hball"
            )

            for n in range(BPC):
                X = work.tile([BS, BS], f32, tag="X")
                nc.sync.dma_start(X[:], wb[n])

                y = work.tile([BS, BS], f32, tag="y")
                nc.vector.tensor_scalar(
                    y[:], X[:], sc255[:, n : n + 1], b255[:, n : n + 1],
                    Alu.mult, Alu.add,
                )
                nc.vector.tensor_scalar(y[:], y[:], 0.0, 255.0, Alu.max, Alu.min)
                # k = floor(y) (y>=0): int round-trip + fix-up, works for
                # either trunc or round-to-nearest conversion semantics
                ki = work.tile([BS, BS], mybir.dt.int32, tag="ki")
                nc.vector.tensor_copy(ki[:], y[:])
                kk = work.tile([BS, BS], f32, tag="k")
                nc.vector.tensor_copy(kk[:], ki[:])
                gt = work.tile([BS, BS], f32, tag="gt")
                nc.vector.tensor_tensor(gt[:], kk[:], y[:], Alu.is_gt)
                nc.vector.tensor_sub(kk[:], kk[:], gt[:])
                tt = work.tile([BS, BS], f32, tag="t")
                nc.vector.tensor_sub(tt[:], y[:], kk[:])

                ea = work.tile([BS, BS], f32, tag="ea")
                eb = work.tile([BS, BS], f32, tag="eb")
                E1 = work.tile([BS, BS], f32, tag="E1")
                E2 = work.tile([BS, BS], f32, tag="E2")
                nc.scalar.activation(ea[:], tt[:], Act.Exp, scale=-TD)
                nc.scalar.activation(eb[:], tt[:], Act.Exp, bias=-TD, scale=TD)
                nc.scalar.activation(E1[:], kk[:], Act.Exp, bias=-TD, scale=-TD)
                nc.scalar.activation(E2[:], kk[:], Act.Exp, bias=-T, scale=TD)

                p1 = work.tile([BS, BS], f32, tag="p1")
                p2 = work.tile([BS, BS], f32, tag="p2")
                s1 = work.tile([BS, BS], f32, tag="s1")
                nc.gpsimd.tensor_mul(p1[:], ea[:], E1[:])
                nc.gpsimd.tensor_mul(p2[:], eb[:], E2[:])
                nc.gpsimd.tensor_add(s1[:], ea[:], eb[:])
                denp = work.tile([BS, BS], f32, tag="denp")
                nc.gpsimd.tensor_add(denp[:], p1[:], p2[:])
                nc.vector.tensor_sub(denp[:], s1[:], denp[:])

                # num (already divided by c1):
                #   A = k*s1 + eb ; B = p1/r - ea + eb
                #   n2 = A + (c3/c1)*B - (c4/c1)*p2
                A = work.tile([BS, BS], f32, tag="A")
                nc.gpsimd.tensor_mul(A[:], kk[:], s1[:])
                nc.gpsimd.tensor_add(A[:], A[:], eb[:])
                B = work.tile([BS, BS], f32, tag="B")
                nc.vector.scalar_tensor_tensor(
                    B[:], p1[:], float(1.0 / r), ea[:], Alu.mult, Alu.subtract
                )
                nc.gpsimd.tensor_add(B[:], B[:], eb[:])
                n2 = work.tile([BS, BS], f32, tag="n2")
                nc.vector.scalar_tensor_tensor(
                    n2[:], B[:], float(c3 / c1), A[:], Alu.mult, Alu.add
                )
                nc.vector.scalar_tensor_tensor(
                    n2[:], p2[:], float(-c4 / c1), n2[:], Alu.mult, Alu.add
                )

                recd = work.tile([BS, BS], f32, tag="recd")
                nc.vector.reciprocal(recd[:], denp[:])
                q = work.tile([BS, BS], f32, tag="q")
                nc.vector.tensor_mul(q[:], n2[:], recd[:])
                wdq = work.tile([BS, BS], f32, tag="wdq")
                nc.vector.tensor_scalar(
                    wdq[:], q[:], dscalev[:, n : n + 1], wminv[:, n : n + 1],
                    Alu.mult, Alu.add,
                )
                nc.sync.dma_start(deq_d[n], wdq[:])

                # ---- entropy stage 1: digit one-hots + histogram matmuls ----
                glbf = work.tile([BS, BS], bf16, tag="glbf")
                grbf = work.tile([BS, BS], bf16, tag="grbf")
                nc.vector.scalar_tensor_tensor(
                    glbf[:], ea[:], 1.0, recd[:], Alu.mult, Alu.mult
                )
                nc.vector.scalar_tensor_tensor(
                    grbf[:], eb[:], 1.0, recd[:], Alu.mult, Alu.mult
                )

                # digits of k (int32): khi = k >> 4, klo = k & 15, as bf16
                # (re-convert from the fixed-up kk, not the raw rounded ki)
                ki2 = work.tile([BS, BS], mybir.dt.int32, tag="ki2")
                nc.vector.tensor_copy(ki2[:], kk[:])
                khi_i = work.tile([BS, BS], mybir.dt.int32, tag="khii")
                klo_i = work.tile([BS, BS], mybir.dt.int32, tag="kloi")
                nc.vector.tensor_scalar(
                    khi_i[:], ki2[:], 4, None, Alu.arith_shift_right
                )
                nc.vector.tensor_scalar(
                    klo_i[:], ki2[:], 15, None, Alu.bitwise_and
                )
                khi = work.tile([BS, BS], bf16, tag="khi")
                klo = work.tile([BS, BS], bf16, tag="klo")
                nc.scalar.copy(khi[:], khi_i[:])
                nc.scalar.copy(klo[:], klo_i[:])

                def _bcast16(tileap):
                    ap = tileap
                    return bass.AP(
                        ap.tensor, ap.offset, [ap.ap[0], [0, 16], ap.ap[1]]
                    )

                olf = ohpool.tile([BS, 16, BS], bf16, tag="olf")
                ohf = ohpool.tile([BS, 16, BS], bf16, tag="ohf")
                nc.vector.tensor_tensor(
                    olf[:], _bcast16(klo[:]), iota_t[:], Alu.is_equal
                )
                nc.vector.tensor_tensor(
                    ohf[:], _bcast16(khi[:]), iota_t[:], Alu.is_equal
                )
                rhsf = ohpool.tile([BS, 2, 16, BS], bf16, tag="rhsf")
                nc.vector.tensor_tensor(
                    rhsf[:, 0], ohf[:], _bcast16(glbf[:]), Alu.mult
                )
                nc.vector.tensor_tensor(
                    rhsf[:, 1], ohf[:], _bcast16(grbf[:]), Alu.mult
                )

                # 128 histogram matmuls: out[b, (s,a)] per column j
                histc = None
                for j in range(BS):
                    if j % 16 == 0:
                        histc = psum_hist.tile([16, 16, 2, 16], f32, tag="hist")
                    nc.tensor.matmul(
                        histc[:, j % 16, :, :],
                        olf[:, :, j],
                        rhsf[:, :, :, j],
                        start=True,
                        stop=True,
                    )
                    if j % 16 == 15:
                        c = j // 16
                        cp = nc.scalar.copy
                        cp(hb_all[:, n, 16 * c : 16 * (c + 1), :, :], histc[:])

            # ---- entropy stage 2: bin^T = sum_(a,b) K-slices, all blocks ----
            binTg = {}
            for h in range(2):
                for g in range(2):
                    binTg[(h, g)] = psum_bin.tile(
                        [BS, 4, BS], f32, tag=f"bt{h}{g}", name=f"bt{h}{g}"
                    )
            for g in range(2):
                for s in range(2):
                    for a in range(16):
                        for h in range(2):
                            lhsT = kmat_t[:, (s * 2 + h) * 16 + a, :]
                            nc.tensor.matmul(
                                binTg[(h, g)][:],
                                lhsT,
                                hb_all[:, 4 * g : 4 * (g + 1), :, s, a],
                                start=(s == 0 and a == 0),
                                stop=(s == 1 and a == 15),
                                skip_group_check=True,
                            )
            binT = [
                (lambda n: [binTg[(0, n // 4)][:, n % 4, :],
                            binTg[(1, n // 4)][:, n % 4, :]])(n)
                for n in range(BPC)
            ]

            # ---- entropy epilogue per block ----
            for n in range(BPC):
                dcol = work.tile([BS, 1], f32, tag="dcol")
                dcol2 = work.tile([BS, 1], f32, tag="dcol2")
                nc.vector.tensor_reduce(
                    dcol[:], binT[n][0], mybir.AxisListType.X, Alu.add
                )
                nc.vector.tensor_reduce(
                    dcol2[:], binT[n][1], mybir.AxisListType.X, Alu.add
                )
                nc.gpsimd.tensor_add(dcol[:], dcol[:], dcol2[:])
                drow = psum_small.tile([BS, BPC], f32, tag="small")
                nc.tensor.matmul(
                    drow[:, 0:1], ones_t[:], dcol[:], start=True, stop=True
                )
                recD = work.tile([BS, 1], f32, tag="recD")
                nc.vector.reciprocal(recD[:], drow[:, 0:1])
                for h in range(2):
                    bprob = work.tile([BS, BS], f32, tag="bprob")
                    nc.scalar.activation(
                        bprob[:], binT[n][h], Act.Copy, scale=recD[:]
                    )
                    lnb = work.tile([BS, BS], f32, tag="lnb")
                    nc.scalar.activation(lnb[:], bprob[:], Act.Ln, bias=EPS)
                    escr = work.tile([BS, BS], f32, tag="escr")
                    nc.vector.scalar_tensor_tensor(
                        escr[:], bprob[:], 1.0, lnb[:], Alu.mult, Alu.mult,
                        accum_out=ent_acc[:, 2 * n + h : 2 * n + h + 1],
                    )

            # total entropy partial: sum columns then partitions (ones-matmul)
            erow = prepool.tile([BS, 1], f32, tag="erow")
            nc.vector.tensor_reduce(erow[:], ent_acc[:], mybir.AxisListType.X, Alu.add)
            etps = psum_small.tile([BS, BPC], f32, tag="small")
            nc.tensor.matmul(
                etps[0:1, 0:1], erow[:], ones_t[:, 0:1], start=True, stop=True
            )
            etot = prepool.tile([1, 1], f32, tag="etot")
            nc.vector.tensor_copy(etot[:], etps[0:1, 0:1])
            nc.sync.dma_start(ent_d.ap(), etot[:])

    _fix_sync_waits(nc)
    return nc


def _get_nc():
    if "nc" not in _cache:
        _cache["nc"] = _build_bass()
    return _cache["nc"]


def _to_blocks(w):
    return (
        w.reshape(NB_R, BS, NB_C, BS).transpose(0, 2, 1, 3).reshape(NB, BS, BS)
    )


def _from_blocks(b):
    return (
        b.reshape(NB_R, NB_C, BS, BS).transpose(0, 2, 1, 3).reshape(NB_R * BS, NB_C * BS)
    )


def kernel(weight, w_min, w_max, _trace=False):
    from concourse.bass_utils import run_bass_kernel_spmd

    weight = np.ascontiguousarray(np.asarray(weight, dtype=np.float32))
    w_min = np.asarray(w_min, dtype=np.float32)
    w_max = np.asarray(w_max, dtype=np.float32)

    blocks = np.ascontiguousarray(_to_blocks(weight))
    iotafb, kdig = _host_tables()

    in_maps = []
    for c in range(N_CORES):
        sl = slice(c * BPC, (c + 1) * BPC)
        in_maps.append(
            {
                "wb": np.ascontiguousarray(blocks[sl]),
                "wmn": np.ascontiguousarray(w_min[sl].reshape(1, BPC)),
                "wmx": np.ascontiguousarray(w_max[sl].reshape(1, BPC)),
                "iotafb": np.ascontiguousarray(iotafb),
                "kdig": np.ascontiguousarray(kdig),
            }
        )

    nc = _get_nc()
    res = run_bass_kernel_spmd(nc, in_maps, list(range(N_CORES)), trace=_trace)

    deq_blocks = np.concatenate(
        [res.results[c]["deq"] for c in range(N_CORES)], axis=0
    )
    dequant = np.ascontiguousarray(_from_blocks(deq_blocks)).astype(np.float32)
    ent_partials = [float(res.results[c]["ent"][0, 0]) for c in range(N_CORES)]
    total_entropy = np.float32(-sum(ent_partials))
    if _trace:
        return (dequant, total_entropy), res
    return dequant, total_entropy


# revision 13
# speedup vs baseline: 1.0019x; 1.0019x over previous
"""Trainium2 Bass kernel for blockwise soft-quantization dequant + codebook entropy.

Problem (nn_BlockwiseQuantizationOptim): weight [1024,1024] f32 split into 64
blocks of 128x128; per-block soft quantization onto 256 uniform levels with
softmax temperature T=100, returning (dequantized weight [1024,1024], total
soft-assignment entropy scalar).

Sharding: block axis across 8 cores (8 blocks/core). No collectives needed —
entropy is a per-block sum; host adds the 8 partial scalars and reassembles
the dequant blocks.

Math (closed forms instead of the naive [*, 256] softmax):
  With q_l = l/255, d = T/255, r = e^-d, the softmax over levels of
  -T|x - q_l| has geometric structure. For y = 255*clip(x_norm,0,1),
  t = y mod 1, k = y - t:
    ea = r^t, eb = r^(1-t), E1 = r^(k+1), E2 = r^(255-k)
    den = ea+eb - (ea*E1 + eb*E2)                    (= Z*(1-r))
    num = c1*(k*(ea+eb) + eb) + c3*(ea*E1/r - ea + eb) - c4*eb*E2
    w_q = (1/255) * num/den, with c1=1/(1-r), c3=r/(1-r)^2,
    c4=(256-255r)/(1-r)^2   (num here folded by 1/c1; see code)
  Entropy bin masses bin[j,l] = sum_i p_l(x[i,j]) reduce to per-column
  weighted histograms over the integer level k:
    HL[j,m] = sum_{i: k=m} ea/Z,  HR[j,m] = sum_{i: k=m} eb/Z
    bin = HL @ KL + HR @ KR,  KL[m,l] = r^(m-l)[m>=l], KR[m,l]=r^(l-1-m)[m<l]
  The histograms are built exactly on the PE: per column j a one-hot
  matrix O[i,m] = [k[i,j]==m] (bf16) is multiplied against (gl,gr).
"""
import sys

sys.path.insert(0, "/opt/trn_rl_repo")

import numpy as np

N_CORES = 8
BS = 128
NB_R = NB_C = 8
NB = NB_R * NB_C            # 64 blocks
BPC = NB // N_CORES         # 8 blocks per core
L = 256
T = 100.0
EPS = 1e-6
DELTA = 1.0 / (L - 1)
TD = T * DELTA              # 100/255

_cache = {}


def _constants():
    r = float(np.exp(np.float64(-TD)))
    c1 = 1.0 / (1.0 - r)
    c3 = r / (1.0 - r) ** 2
    c4 = (256.0 - 255.0 * r) / (1.0 - r) ** 2
    return r, c1, c3, c4


def _host_tables():
    """Digit-scheme tables:
    iotafb [128, 16, 128] bf16: value = b at [:, b, :]
    kdig   [16, 64, 128] bf16: kdig[b, (s*2+h)*16+a, ll] = K_s[16a+b, 128h+ll]
    """
    import ml_dtypes

    r, c1, _, _ = _constants()
    bf = ml_dtypes.bfloat16
    iotafb = np.broadcast_to(
        np.arange(16, dtype=np.float32)[None, :, None], (BS, 16, BS)
    ).astype(bf)
    m = np.arange(L, dtype=np.float64)[:, None]
    l = np.arange(L, dtype=np.float64)[None, :]
    KL = np.where(m >= l, np.exp(-TD * (m - l)), 0.0) * (1.0 - r)
    KR = np.where(m <= l - 1, np.exp(-TD * (l - 1 - m)), 0.0) * (1.0 - r)
    Ks = [KL.astype(np.float32), KR.astype(np.float32)]
    kdig = np.empty((16, 64, BS), dtype=np.float32)
    for s in range(2):
        for h in range(2):
            for a in range(16):
                for b_ in range(16):
                    kdig[b_, (s * 2 + h) * 16 + a, :] = Ks[s][
                        16 * a + b_, 128 * h : 128 * (h + 1)
                    ]
    return iotafb, kdig.astype(bf)


def _build_bass():
    import bass_rust
    import concourse.bass as bass
    import concourse.tile as tile
    import concourse.mybir as mybir
    from concourse.vector_clock import ScopedClock

    f32 = mybir.dt.float32
    bf16 = mybir.dt.bfloat16
    Alu = mybir.AluOpType
    Act = mybir.ActivationFunctionType
    r, c1, c3, c4 = _constants()

    # walrus's codegen accepts at most ONE sync wait per instruction.  Tile
    # emits several.  Fix up after scheduling: (a) drop waits on the
    # instruction's own engine semaphore — those engines complete in order,
    # so program order implies them; (b) hoist remaining extra waits onto
    # single-wait NoOps inserted just before the instruction on the same
    # engine stream.
    _self_pref = {
        "EngineType.Activation": "Activation_",
        "EngineType.DVE": "DVE_",
        "EngineType.PE": "PE_",
        "EngineType.SP": "SP_",
    }

    def _fix_sync_waits(nc):
        ctr = [0]
        for bb in nc.m.functions[0].blocks:
            out = []
            changed = False
            for inst in bb.instructions:
                si = inst.sync_info
                waits = list(si.on_wait) if si is not None and si.on_wait else []
                orig_n = len(waits)
                if len(waits) > 1:
                    pref = _self_pref.get(str(inst.engine))
                    if pref is not None:
                        kept = [
                            w
                            for w in waits
                            if not (
                                w.sync_type == "semaphore"
                                and w.ant_name
                                and w.ant_name.startswith(pref)
                            )
                        ]
                        waits = kept if kept else waits[-1:]
                if len(waits) > 1:
                    for w in waits[:-1]:
                        nop = bass_rust.InstNoOp(
                            name=f"I-wsplit-{ctr[0]}", ins=[], outs=[]
                        )
                        ctr[0] += 1
                        nop.engine = inst.engine
                        nop.sync_info = bass_rust.SyncInfo(
                            on_wait=[w], on_update=[]
                        )
                        out.append(nop)
                    waits = waits[-1:]
                if si is not None and len(waits) != orig_n:
                    inst.sync_info = bass_rust.SyncInfo(
                        on_wait=waits,
                        on_update=list(si.on_update) if si.on_update else [],
                    )
                    changed = True
                out.append(inst)
            if changed or len(out) != len(bb.instructions):
                bb.instructions = out

    nc = bass.Bass("TRN2", target_bir_lowering=False, debug=False)

    # register const APs for activation biases (only 0.0/1.0 pre-registered)
    for _cv in (-TD, -T, EPS):
        _ct = nc.alloc_sbuf_tensor(f"constx-f32-{_cv}", [128, 1], f32)
        nc.gpsimd.memset(_ct.ap(), _cv)
        nc.const_aps.aps[(f32, _cv)] = _ct.ap()
    nc.all_engine_barrier()

    wb = nc.dram_tensor("wb", [BPC, BS, BS], f32, kind="ExternalInput")
    wmn = nc.dram_tensor("wmn", [1, BPC], f32, kind="ExternalInput")
    wmx = nc.dram_tensor("wmx", [1, BPC], f32, kind="ExternalInput")
    iota_d = nc.dram_tensor("iotafb", [BS, 16, BS], bf16, kind="ExternalInput")
    kmat_d = nc.dram_tensor("kdig", [16, 64, BS], bf16, kind="ExternalInput")
    deq_d = nc.dram_tensor("deq", [BPC, BS, BS], f32, kind="ExternalOutput")
    ent_d = nc.dram_tensor("ent", [1, 1], f32, kind="ExternalOutput")

    with tile.TileContext(nc) as tc:
        with (
            tc.tile_pool(name="const", bufs=1) as cpool,
            tc.tile_pool(name="pre", bufs=1) as prepool,
            tc.tile_pool(name="work", bufs=3) as work,
            tc.tile_pool(name="onehot", bufs=3) as ohpool,
            tc.tile_pool(name="hbufs", bufs=1) as hbpool,
            tc.tile_pool(name="psum_hist", bufs=2, space="PSUM") as psum_hist,
            tc.tile_pool(name="psum_bin", bufs=1, space="PSUM") as psum_bin,
            tc.tile_pool(name="psum_small", bufs=2, space="PSUM") as psum_small,
        ):
            iota_t = cpool.tile([BS, 16, BS], bf16, tag="iota")
            nc.sync.dma_start(iota_t[:], iota_d.ap())
            kmat_t = cpool.tile([16, 64, BS], bf16, tag="kmat")
            nc.sync.dma_start(kmat_t[:], kmat_d.ap())
            ones_t = cpool.tile([BS, BS], f32, tag="ones")
            nc.vector.memset(ones_t[:], 1.0)

            # ---- per-block scalar prep, broadcast to all 128 partitions ----
            # (partition broadcast = ones-matmul on the PE)
            mnb = prepool.tile([1, BPC], f32, tag="mnb")
            mxb = prepool.tile([1, BPC], f32, tag="mxb")
            nc.sync.dma_start(mnb[:], wmn.ap())
            nc.sync.dma_start(mxb[:], wmx.ap())
            bc_mn = prepool.tile([BS, BPC], f32, tag="bcmn")
            bc_mx = prepool.tile([BS, BPC], f32, tag="bcmx")
            bc_ps = psum_small.tile([BS, BPC], f32, tag="small")
            nc.tensor.matmul(bc_ps[:], ones_t[0:1, :], mnb[:], start=True, stop=True)
            nc.vector.tensor_copy(bc_mn[:], bc_ps[:])
            bc_ps2 = psum_small.tile([BS, BPC], f32, tag="small")
            nc.tensor.matmul(bc_ps2[:], ones_t[0:1, :], mxb[:], start=True, stop=True)
            nc.vector.tensor_copy(bc_mx[:], bc_ps2[:])

            tmp_a = prepool.tile([BS, BPC], f32, tag="tmpa")
            wminv = prepool.tile([BS, BPC], f32, tag="wminv")
            wmaxv = prepool.tile([BS, BPC], f32, tag="wmaxv")
            scalev = prepool.tile([BS, BPC], f32, tag="scalev")
            invv = prepool.tile([BS, BPC], f32, tag="invv")
            sc255 = prepool.tile([BS, BPC], f32, tag="sc255")
            b255 = prepool.tile([BS, BPC], f32, tag="b255")
            dscalev = prepool.tile([BS, BPC], f32, tag="dscalev")

            nc.vector.tensor_scalar(tmp_a[:], bc_mx[:], EPS, None, Alu.subtract)
            nc.vector.tensor_tensor(wminv[:], bc_mn[:], tmp_a[:], Alu.min)
            nc.vector.tensor_scalar(tmp_a[:], wminv[:], EPS, None, Alu.add)
            nc.vector.tensor_tensor(wmaxv[:], bc_mx[:], tmp_a[:], Alu.max)
            nc.vector.tensor_sub(scalev[:], wmaxv[:], wminv[:])
            nc.vector.tensor_scalar(tmp_a[:], scalev[:], EPS, None, Alu.add)
            nc.vector.reciprocal(invv[:], tmp_a[:])
            nc.vector.tensor_scalar(sc255[:], invv[:], 255.0, None, Alu.mult)
            nc.vector.tensor_scalar(tmp_a[:], wminv[:], -255.0, None, Alu.mult)
            nc.vector.tensor_mul(b255[:], tmp_a[:], invv[:])
            nc.vector.tensor_scalar(dscalev[:], scalev[:], float(DELTA), None, Alu.mult)

            # entropy accumulator [128, 2*BPC], one column per (block, half)
            ent_acc = prepool.tile([BS, 2 * BPC], f32, tag="entacc")
            nc.vector.memset(ent_acc[:], 0.0)

            hb_all = hbpool.tile(
                [16, BPC, BS, 2, 16], bf16, tag="hball", name="hball"
            )

            for n in range(BPC):
                X = work.tile([BS, BS], f32, tag="X")
                nc.sync.dma_start(X[:], wb[n])

                y = work.tile([BS, BS], f32, tag="y")
                nc.vector.tensor_scalar(
                    y[:], X[:], sc255[:, n : n + 1], b255[:, n : n + 1],
                    Alu.mult, Alu.add,
                )
                nc.vector.tensor_scalar(y[:], y[:], 0.0, 255.0, Alu.max, Alu.min)
                # k = floor(y) (y>=0): int round-trip + fix-up, works for
                # either trunc or round-to-nearest conversion semantics
                ki = work.tile([BS, BS], mybir.dt.int32, tag="ki")
                nc.vector.tensor_copy(ki[:], y[:])
                kk = work.tile([BS, BS], f32, tag="k")
                nc.vector.tensor_copy(kk[:], ki[:])
                gt = work.tile([BS, BS], f32, tag="gt")
                nc.vector.tensor_tensor(gt[:], kk[:], y[:], Alu.is_gt)
                nc.vector.tensor_sub(kk[:], kk[:], gt[:])
                tt = work.tile([BS, BS], f32, tag="t")
                nc.vector.tensor_sub(tt[:], y[:], kk[:])

                ea = work.tile([BS, BS], f32, tag="ea")
                eb = work.tile([BS, BS], f32, tag="eb")
                E1 = work.tile([BS, BS], f32, tag="E1")
                E2 = work.tile([BS, BS], f32, tag="E2")
                nc.scalar.activation(ea[:], tt[:], Act.Exp, scale=-TD)
                nc.scalar.activation(eb[:], tt[:], Act.Exp, bias=-TD, scale=TD)
                nc.scalar.activation(E1[:], kk[:], Act.Exp, bias=-TD, scale=-TD)
                nc.scalar.activation(E2[:], kk[:], Act.Exp, bias=-T, scale=TD)

                p1 = work.tile([BS, BS], f32, tag="p1")
                p2 = work.tile([BS, BS], f32, tag="p2")
                s1 = work.tile([BS, BS], f32, tag="s1")
                nc.gpsimd.tensor_mul(p1[:], ea[:], E1[:])
                nc.gpsimd.tensor_mul(p2[:], eb[:], E2[:])
                nc.gpsimd.tensor_add(s1[:], ea[:], eb[:])
                denp = work.tile([BS, BS], f32, tag="denp")
                nc.gpsimd.tensor_add(denp[:], p1[:], p2[:])
                nc.vector.tensor_sub(denp[:], s1[:], denp[:])

                # num (already divided by c1):
                #   A = k*s1 + eb ; B = p1/r - ea + eb
                #   n2 = A + (c3/c1)*B - (c4/c1)*p2
                A = work.tile([BS, BS], f32, tag="A")
                nc.gpsimd.tensor_mul(A[:], kk[:], s1[:])
                nc.gpsimd.tensor_add(A[:], A[:], eb[:])
                B = work.tile([BS, BS], f32, tag="B")
                nc.vector.scalar_tensor_tensor(
                    B[:], p1[:], float(1.0 / r), ea[:], Alu.mult, Alu.subtract
                )
                nc.gpsimd.tensor_add(B[:], B[:], eb[:])
                n2 = work.tile([BS, BS], f32, tag="n2")
                nc.vector.scalar_tensor_tensor(
                    n2[:], B[:], float(c3 / c1), A[:], Alu.mult, Alu.add
                )
                nc.vector.scalar_tensor_tensor(
                    n2[:], p2[:], float(-c4 / c1), n2[:], Alu.mult, Alu.add
                )

                recd = work.tile([BS, BS], f32, tag="recd")
                nc.vector.reciprocal(recd[:], denp[:])
                q = work.tile([BS, BS], f32, tag="q")
                nc.vector.tensor_mul(q[:], n2[:], recd[:])
                wdq = work.tile([BS, BS], f32, tag="wdq")
                nc.vector.tensor_scalar(
                    wdq[:], q[:], dscalev[:, n : n + 1], wminv[:, n : n + 1],
                    Alu.mult, Alu.add,
                )
                nc.sync.dma_start(deq_d[n], wdq[:])

                # ---- entropy stage 1: digit one-hots + histogram matmuls ----
                glbf = work.tile([BS, BS], bf16, tag="glbf")
                grbf = work.tile([BS, BS], bf16, tag="grbf")
                nc.vector.scalar_tensor_tensor(
                    glbf[:], ea[:], 1.0, recd[:], Alu.mult, Alu.mult
                )
                nc.vector.scalar_tensor_tensor(
                    grbf[:], eb[:], 1.0, recd[:], Alu.mult, Alu.mult
                )

                # digits of k (int32): khi = k >> 4, klo = k & 15, as bf16
                # (re-convert from the fixed-up kk, not the raw rounded ki)
                ki2 = work.tile([BS, BS], mybir.dt.int32, tag="ki2")
                nc.vector.tensor_copy(ki2[:], kk[:])
                khi_i = work.tile([BS, BS], mybir.dt.int32, tag="khii")
                klo_i = work.tile([BS, BS], mybir.dt.int32, tag="kloi")
                nc.vector.tensor_scalar(
                    khi_i[:], ki2[:], 4, None, Alu.arith_shift_right
                )
                nc.vector.tensor_scalar(
                    klo_i[:], ki2[:], 15, None, Alu.bitwise_and
                )
                khi = work.tile([BS, BS], bf16, tag="khi")
                klo = work.tile([BS, BS], bf16, tag="klo")
                nc.scalar.copy(khi[:], khi_i[:])
                nc.scalar.copy(klo[:], klo_i[:])

                def _bcast16(tileap):
                    ap = tileap
                    return bass.AP(
                        ap.tensor, ap.offset, [ap.ap[0], [0, 16], ap.ap[1]]
                    )

                olf = ohpool.tile([BS, 16, BS], bf16, tag="olf")
                ohf = ohpool.tile([BS, 16, BS], bf16, tag="ohf")
                nc.vector.tensor_tensor(
                    olf[:], _bcast16(klo[:]), iota_t[:], Alu.is_equal
                )
                nc.vector.tensor_tensor(
                    ohf[:], _bcast16(khi[:]), iota_t[:], Alu.is_equal
                )
                rhsf = ohpool.tile([BS, 2, 16, BS], bf16, tag="rhsf")
                nc.vector.tensor_tensor(
                    rhsf[:, 0], ohf[:], _bcast16(glbf[:]), Alu.mult
                )
                nc.vector.tensor_tensor(
                    rhsf[:, 1], ohf[:], _bcast16(grbf[:]), Alu.mult
                )

                # 128 histogram matmuls: out[b, (s,a)] per column j
                histc = None
                for j in range(BS):
                    if j % 16 == 0:
                        histc = psum_hist.tile([16, 16, 2, 16], f32, tag="hist")
                    nc.tensor.matmul(
                        histc[:, j % 16, :, :],
                        olf[:, :, j],
                        rhsf[:, :, :, j],
                        start=True,
                        stop=True,
                    )
                    if j % 16 == 15:
                        c = j // 16
                        cp = nc.scalar.copy
                        cp(hb_all[:, n, 16 * c : 16 * (c + 1), :, :], histc[:])

            # ---- entropy stage 2: bin^T = sum_(a,b) K-slices, all blocks ----
            binTg = {}
            for h in range(2):
                for g in range(2):
                    binTg[(h, g)] = psum_bin.tile(
                        [BS, 4, BS], f32, tag=f"bt{h}{g}", name=f"bt{h}{g}"
                    )
            for g in range(2):
                for s in range(2):
                    for a in range(16):
                        for h in range(2):
                            lhsT = kmat_t[:, (s * 2 + h) * 16 + a, :]
                            nc.tensor.matmul(
                                binTg[(h, g)][:],
                                lhsT,
                                hb_all[:, 4 * g : 4 * (g + 1), :, s, a],
                                start=(s == 0 and a == 0),
                                stop=(s == 1 and a == 15),
                                skip_group_check=True,
                            )
            binT = [
                (lambda n: [binTg[(0, n // 4)][:, n % 4, :],
                            binTg[(1, n // 4)][:, n % 4, :]])(n)
                for n in range(BPC)
            ]

            # ---- entropy epilogue per block ----
            for n in range(BPC):
                dcol = work.tile([BS, 1], f32, tag="dcol")
                dcol2 = work.tile([BS, 1], f32, tag="dcol2")
                nc.vector.tensor_reduce(
                    dcol[:], binT[n][0], mybir.AxisListType.X, Alu.add
                )
                nc.vector.tensor_reduce(
                    dcol2[:], binT[n][1], mybir.AxisListType.X, Alu.add
                )
                nc.gpsimd.tensor_add(dcol[:], dcol[:], dcol2[:])
                drow = psum_small.tile([BS, BPC], f32, tag="small")
                nc.tensor.matmul(
                    drow[:, 0:1], ones_t[:], dcol[:], start=True, stop=True
                )
                recD = work.tile([BS, 1], f32, tag="recD")
                nc.vector.reciprocal(recD[:], drow[:, 0:1])
                for h in range(2):
                    bprob = work.tile([BS, BS], f32, tag="bprob")
                    nc.scalar.activation(
                        bprob[:], binT[n][h], Act.Copy, scale=recD[:]
                    )
                    lnb = work.tile([BS, BS], f32, tag="lnb")
                    nc.scalar.activation(lnb[:], bprob[:], Act.Ln, bias=EPS)
                    escr = work.tile([BS, BS], f32, tag="escr")
                    nc.vector.scalar_tensor_tensor(
                        escr[:], bprob[:], 1.0, lnb[:], Alu.mult, Alu.mult,
                        accum_out=ent_acc[:, 2 * n + h : 2 * n + h + 1],
                    )

            # total entropy partial: sum columns then partitions (ones-matmul)
            erow = prepool.tile([BS, 1], f32, tag="erow")
            nc.vector.tensor_reduce(erow[:], ent_acc[:], mybir.AxisListType.X, Alu.add)
            etps = psum_small.tile([BS, BPC], f32, tag="small")
            nc.tensor.matmul(
                etps[0:1, 0:1], erow[:], ones_t[:, 0:1], start=True, stop=True
            )
            etot = prepool.tile([1, 1], f32, tag="etot")
            nc.vector.tensor_copy(etot[:], etps[0:1, 0:1])
            nc.sync.dma_start(ent_d.ap(), etot[:])

    _fix_sync_waits(nc)
    return nc


def _get_nc():
    if "nc" not in _cache:
        _cache["nc"] = _build_bass()
    return _cache["nc"]


def _to_blocks(w):
    return (
        w.reshape(NB_R, BS, NB_C, BS).transpose(0, 2, 1, 3).reshape(NB, BS, BS)
    )


def _from_blocks(b):
    return (
        b.reshape(NB_R, NB_C, BS, BS).transpose(0, 2, 1, 3).reshape(NB_R * BS, NB_C * BS)
    )


def kernel(weight, w_min, w_max, _trace=False):
    from concourse.bass_utils import run_bass_kernel_spmd

    weight = np.ascontiguousarray(np.asarray(weight, dtype=np.float32))
    w_min = np.asarray(w_min, dtype=np.float32)
    w_max = np.asarray(w_max, dtype=np.float32)

    blocks = np.ascontiguousarray(_to_blocks(weight))
    iotafb, kdig = _host_tables()

    in_maps = []
    for c in range(N_CORES):
        sl = slice(c * BPC, (c + 1) * BPC)
        in_maps.append(
            {
                "wb": np.ascontiguousarray(blocks[sl]),
                "wmn": np.ascontiguousarray(w_min[sl].reshape(1, BPC)),
                "wmx": np.ascontiguousarray(w_max[sl].reshape(1, BPC)),
                "iotafb": np.ascontiguousarray(iotafb),
                "kdig": np.ascontiguousarray(kdig),
            }
        )

    nc = _get_nc()
    res = run_bass_kernel_spmd(nc, in_maps, list(range(N_CORES)), trace=_trace)

    deq_blocks = np.concatenate(
        [res.results[c]["deq"] for c in range(N_CORES)], axis=0
    )
    dequant = np.ascontiguousarray(_from_blocks(deq_blocks)).astype(np.float32)
    ent_partials = [float(res.results[c]["ent"][0, 0]) for c in range(N_CORES)]
    total_entropy = np.float32(-sum(ent_partials))
    if _trace:
        return (dequant, total_entropy), res
    return dequant, total_entropy
